# revision 6
# baseline (speedup 1.0000x reference)
"""Trainium2 Bass kernel: banded additive attention (window 64).

reference semantics (B=4, T=1024, D=512, U=32, WIDTH=64):
  q = x @ Wt ; k = x @ Wx
  e[b,t,j] = exp(Wa . tanh(q[b,t]+k[b,j]+bh) + ba) for j in [t-32, t+31]
  a = e / (sum_j e + 1e-7) ; v = a @ x

Sharding: 8 NeuronCores = (batch b, T-half). Each core computes 512 query
rows with a 32-row halo; weights replicated. Inputs are packed/cast on the
host into SBUF-shaped blocks so each needs a single DMA.

Per-core pipeline (Tile-scheduled), v2 layout: the 512 query columns are
processed as two independent 256-column halves so that half B's tanh
stream (ACT is the bottleneck engine) overlaps half A's serial
exp->shear->value tail.

  0. PE-warmup matmuls (p-state ramp) + input DMAs on one HWDGE queue in
     dependency order (weights, x^T chunks, biases, value-side x last).
  1. PE projections kT (contract D in 4 chunks, chunk-pipelined with the
     DMAs) -> ACT/Pool copies to SBUF -> 4 shifted-replication matmuls
     (diagonal layout k4[32g+u, c] = kT[u, c+g]) -> DVE copy to SBUF.
     q4 projection against host-replicated Wt; Pool copies it out; bh is
     folded into the tanh bias operand.
  2. Per half h (t in [256h, 256h+256)): DVE add q4+k4 (diag layout) ->
     ACT tanh (bias=bh) -> 16 accumulating PE matmuls with sliced wide-Wa
     lhsT -> E[64, 256] PSUM (+rank-32 edge-mask accumulations).
  3. ACT exp (bias=ba) writes B0 in sigmaA order; 2-stage radix-8 shear
     butterfly (PE shift-matmuls, permutations ride on DVE/Pool strided
     copies) -> banded layout Bsb.
  4. Value: per 64-row block, Bsb-block.T @ xe-block (xe carries a ones
     column so the softmax denominator falls out of the same matmul);
     DVE reciprocal; DVE/Pool per-partition scale; one 3D DMA out per
     half.
"""
import os
import sys

sys.path.insert(0, "/opt/trn_rl_repo")

import numpy as np
import ml_dtypes  # noqa: E402
import concourse.bass as bass  # noqa: E402
import concourse.mybir as mybir  # noqa: E402
from concourse import bacc, tile  # noqa: E402
from concourse.ap import AP  # noqa: E402
from concourse.bass_utils import run_bass_kernel_spmd  # noqa: E402

F32 = mybir.dt.float32
BF16 = mybir.dt.bfloat16
ActFn = mybir.ActivationFunctionType

B, T, D, U = 4, 1024, 512, 32
WIDTH = 64
EPS = 1e-7
T_LOC = 512
TH = 256  # half width
HALO = 576
NBLK = 8
NCORES = 8

_CDT = BF16 if os.environ.get("ATTN_CDT", "bf16") == "bf16" else F32
_NWARM = int(os.environ.get("ATTN_WARM", "4"))


def _np_dt(cdt):
    return ml_dtypes.bfloat16 if cdt == BF16 else np.float32


def _emit(nc, tc, cdt, xt, xe, wws, mbb, vout):
    from contextlib import ExitStack
    ctx = ExitStack()
    with ctx:
        cpool = ctx.enter_context(tc.tile_pool(name="consts", bufs=1))
        work = ctx.enter_context(tc.tile_pool(name="work", bufs=1))
        tpool = ctx.enter_context(tc.tile_pool(name="tanh", bufs=2))
        hpool = ctx.enter_context(tc.tile_pool(name="half", bufs=2))
        opool = ctx.enter_context(tc.tile_pool(name="outs", bufs=2))
        rpool = ctx.enter_context(tc.tile_pool(name="rcols", bufs=4))

        # ---------- Phase 0: ACT table preload, PE warmup, DMAs ----------
        dummy = cpool.tile([1, 1], F32, tag="dummy")
        nc.vector.memset(dummy[:], 0.0)
        nc.scalar.activation(dummy[:], dummy[:], ActFn.Exp)

        wws_sb = cpool.tile([128, 1147], cdt, tag="wws_sb")
        xtc = [cpool.tile([128, HALO], cdt, tag=f"xt{c}", name=f"xt{c}")
               for c in range(4)]
        mbb_sb = cpool.tile([128, 2], F32, tag="mbb_sb")
        xe_all = cpool.tile([128, 8 * 513], cdt, tag="xe_all")
        # One HWDGE queue: transfer order == need order (xe only for value).
        nc.sync.dma_start(wws_sb[:, 0:640], wws[:, 0:640])
        for c in range(4):
            nc.sync.dma_start(xtc[c][:], xt[:, HALO * c:HALO * (c + 1)])
        nc.sync.dma_start(wws_sb[:, 640:1147], wws[:, 640:1147])
        nc.sync.dma_start(mbb_sb[:], mbb[:])
        nc.sync.dma_start(xe_all[:], xe[:])

        w_all = wws_sb[:, 0:640]
        wa_sb = wws_sb[:, 640:764]
        # sh: [128, 255] 0/1 band, sh[k, c] = (c == k + 127)
        sh_sb = wws_sb[:, 764:1019]
        # edge-mask rank-32 factors: R_lo/R_hi [32, 32] in cdt
        rlo_sb = wws_sb[0:32, 1019:1051]
        rhi_sb = wws_sb[0:32, 1051:1083]
        ba_sb = mbb_sb[0:64, 0:1]
        bh4_sb = mbb_sb[:, 1:2]

        if _NWARM:
            warm = cpool.tile([128, 512], cdt, tag="warm")
            nc.vector.memset(warm[:], 0.0)
            with tc.tile_pool(name="wps", bufs=1, space="PSUM") as wps:
                wp = wps.tile([128, 512], F32, tag="wp")
                for i in range(_NWARM):
                    nc.tensor.matmul(wp[:], warm[:, 0:128], warm[:],
                                     start=True, stop=True)

        # ---------- Phase 1: projections ----------
        with tc.tile_pool(name="pp1", bufs=1, space="PSUM") as pp1:
            kTa_ps = pp1.tile([U, 512], F32, tag="kTa")
            kTb_ps = pp1.tile([U, 64], F32, tag="kTb")
            for c in range(4):
                nc.tensor.matmul(kTa_ps[:],
                                 w_all[:, 512 + 32 * c:512 + 32 * c + 32],
                                 xtc[c][:, 0:512],
                                 start=(c == 0), stop=(c == 3))
            for c in range(4):
                nc.tensor.matmul(kTb_ps[:],
                                 w_all[:, 512 + 32 * c:512 + 32 * c + 32],
                                 xtc[c][:, 512:576],
                                 start=(c == 0), stop=(c == 3))
            # q4 directly: lhsT = W4t chunks (Wt replicated x4 in M)
            q4_ps = pp1.tile([128, T_LOC], F32, tag="q4_ps")
            for c in range(4):
                nc.tensor.matmul(q4_ps[:], w_all[:, 128 * c:128 * c + 128],
                                 xtc[c][:, 32:32 + T_LOC],
                                 start=(c == 0), stop=(c == 3))
            q4s = work.tile([128, T_LOC], cdt, tag="q4s")
            nc.scalar.copy(q4s[:], q4_ps[:])
            # k4[32g+u, c] = kT[u, c+g]: stage kT to SBUF (ACT+DVE), then
            # 4 shifted-rep matmuls (lhsT = sh slices) accumulate in PSUM
            kT_sb = work.tile([U, HALO], cdt, tag="kT_sb")
            nc.scalar.copy(kT_sb[:, 0:512], kTa_ps[:])
            nc.vector.tensor_copy(kT_sb[:, 512:576], kTb_ps[:])
            k4_ps = pp1.tile([128, 1024], F32, tag="k4_ps")
            for g in range(4):
                lhsT = sh_sb[0:32, 127 - 32 * g:255 - 32 * g]
                nc.tensor.matmul(k4_ps[:, 0:512],
                                 lhsT, kT_sb[:, g:g + 512],
                                 start=(g == 0), stop=(g == 3))
            for g in range(4):
                lhsT = sh_sb[0:32, 127 - 32 * g:255 - 32 * g]
                nc.tensor.matmul(k4_ps[:, 512:573],
                                 lhsT, kT_sb[:, 512 + g:573 + g],
                                 start=(g == 0), stop=(g == 3))
            k4 = work.tile([128, HALO], cdt, tag="k4")
            nc.vector.tensor_copy(k4[:, 0:320], k4_ps[:, 0:320])
            nc.vector.tensor_copy(k4[:, 320:573], k4_ps[:, 320:573])

        # ---------- Phase 2+3+4: two independent 256-col halves ----------
        spool = ctx.enter_context(
            tc.tile_pool(name="escore", bufs=2, space="PSUM"))
        stpool = ctx.enter_context(
            tc.tile_pool(name="stage", bufs=2, space="PSUM"))
        vpool = ctx.enter_context(
            tc.tile_pool(name="vpsum", bufs=2, space="PSUM"))

        BATCHES = ([(0, 2), (2, 6), (8, 8)], [(0, 8), (8, 8)])
        for h in range(2):
            E_ps = spool.tile([64, TH], F32, tag="E")
            for (s0, nsl) in BATCHES[h]:
                tin = tpool.tile([128, TH * nsl], cdt, tag="tin")
                k4ap = AP(k4[:].tensor, 4 * s0 + TH * h,
                          [[HALO, 128], [4, nsl], [1, TH]])
                q4ap = AP(q4s[:].tensor, TH * h,
                          [[T_LOC, 128], [0, nsl], [1, TH]])
                nc.vector.tensor_add(
                    tin[:].rearrange("p (a t) -> p a t", a=nsl),
                    q4ap, k4ap)
                tout = tpool.tile([128, TH * nsl], cdt, tag="tout")
                nc.scalar.activation(tout[:], tin[:], ActFn.Tanh,
                                     bias=bh4_sb)
                for j in range(nsl):
                    r = s0 + j
                    nc.tensor.matmul(E_ps[:],
                                     wa_sb[:, 60 - 4 * r:124 - 4 * r],
                                     tout[:, TH * j:TH * j + TH],
                                     start=(r == 0), stop=False)
            # edge mask as rank-32 accumulation: E += -30 on invalid j
            if h == 0:
                nc.tensor.matmul(E_ps[:, 0:32], sh_sb[0:32, 127:191],
                                 rlo_sb, start=False, stop=True)
            else:
                nc.tensor.matmul(E_ps[:, 224:256], sh_sb[0:32, 95:159],
                                 rhi_sb, start=False, stop=True)

            # ---- exp straight to B0 in half-local sigmaA order ----
            # t = 64m + 8a + b (m<4); sigmaA col = 32b + 8m + a
            B0 = hpool.tile([128, TH], cdt, tag="B0")
            nc.gpsimd.memset(B0[64:128, :], 0.0)
            b0_out = AP(B0[:].tensor, 0,
                        [[TH, 64], [32, 8], [8, 4], [1, 8]])
            e_in = AP(E_ps[:].tensor, 0,
                      [[TH, 64], [1, 8], [64, 4], [8, 8]])
            nc.scalar.activation(b0_out, e_in, ActFn.Exp, bias=ba_sb)

            # ---- shear butterfly (radix 8 x 8 on t mod 64) ----
            P1 = stpool.tile([128, TH], F32, tag="stage")
            for b in range(8):
                nc.tensor.matmul(P1[:, 32 * b:32 * b + 32],
                                 sh_sb[:, 127 - b:255 - b],
                                 B0[:, 32 * b:32 * b + 32],
                                 start=True, stop=True)
            # S1 col = 32a + 8m + b  <-  P1 col = 32b + 8m + a
            S1 = hpool.tile([128, TH], cdt, tag="S1")
            s1_out = AP(S1[:].tensor, 0,
                        [[TH, 128], [32, 8], [8, 4], [1, 8]])
            p1_in = AP(P1[:].tensor, 0,
                       [[TH, 128], [1, 8], [8, 4], [32, 8]])
            nc.vector.tensor_copy(s1_out, p1_in)
            P2 = stpool.tile([128, TH], F32, tag="stage")
            for a in range(8):
                nc.tensor.matmul(P2[:, 32 * a:32 * a + 32],
                                 sh_sb[:, 127 - 8 * a:255 - 8 * a],
                                 S1[:, 32 * a:32 * a + 32],
                                 start=True, stop=True)
            # Bsb col = 64m + 8a + b  <-  P2 col = 32a + 8m + b
            Bsb = hpool.tile([128, TH], cdt, tag="Bsb")
            bsb_out = AP(Bsb[:].tensor, 0,
                         [[TH, 128], [64, 4], [8, 8], [1, 8]])
            p2_in = AP(P2[:].tensor, 0,
                       [[TH, 128], [8, 4], [32, 8], [1, 8]])
            nc.vector.tensor_copy(bsb_out, p2_in)

            # ---- value + normalize; one 3D DMA out per half ----
            vo = opool.tile([128, 1024], vout.dtype, tag="vo")
            vden = vpool.tile([128, 2], F32, tag="vden")
            for q in range(2):
                vp = vpool.tile([128, 512], F32, tag="vp")
                for hh in range(2):
                    ml = 2 * q + hh
                    m = 4 * h + ml
                    lhsT = Bsb[:, 64 * ml:64 * ml + 64]
                    rhs = xe_all[:, 513 * m:513 * m + 513]
                    nc.tensor.matmul(vp[64 * hh:64 * hh + 64, :],
                                     lhsT, rhs[:, 0:512],
                                     start=True, stop=True)
                    nc.tensor.matmul(vden[64 * hh:64 * hh + 64, q:q + 1],
                                     lhsT, rhs[:, 512:513],
                                     start=True, stop=True)
                # EPS dropped: s >= 64*exp(-~5) makes 1e-7 negligible
                rcol = rpool.tile([128, 1], F32, tag="rcol")
                nc.vector.reciprocal(rcol[:], vden[:, q:q + 1])
                nc.vector.tensor_scalar_mul(vo[:, 512 * q:512 * q + 512],
                                            vp[:], rcol[:])
            dst = AP(vout[:].tensor, TH * h * D,
                     [[D, 128], [128 * D, 2], [1, D]])
            src = AP(vo[:].tensor, 0, [[1024, 128], [512, 2], [1, 512]])
            nc.sync.dma_start(dst, src)


def build_nc(cdt=_CDT):
    nc = bacc.Bacc("TRN2", target_bir_lowering=False)
    xt = nc.dram_tensor("xt", [128, 4 * HALO], cdt, kind="ExternalInput")
    xe = nc.dram_tensor("xe", [128, 8 * 513], cdt, kind="ExternalInput")
    wws = nc.dram_tensor("wws", [128, 1147], cdt, kind="ExternalInput")
    mbb = nc.dram_tensor("mbb", [128, 2], F32, kind="ExternalInput")
    vout = nc.dram_tensor("v", [T_LOC, D], _CDT if os.environ.get("ATTN_VOUT", "bf16") == "bf16" else F32, kind="ExternalOutput")
    with tile.TileContext(nc) as tc:
        _emit(nc, tc, cdt, xt, xe, wws, mbb, vout)
    nc.compile()
    return nc


# ---------------- host-side prep ----------------

def prep_core_inputs(x, Wt, Wx, bh, Wa, ba, core, cdt=_CDT):
    ndt = _np_dt(cdt)
    b, half = core // 2, core % 2
    t0 = half * T_LOC
    lo, hi = t0 - 32, t0 + 544
    pad_lo, pad_hi = max(0, -lo), max(0, hi - T)
    xs = x[b, max(0, lo):min(T, hi), :]
    x_halo = np.pad(xs, ((pad_lo, pad_hi), (0, 0)))     # [576, 512]

    # xt: [128, 4*576], chunk c = x_halo[:, 128c:128c+128].T
    xt = np.empty((128, 4 * HALO), np.float32)
    for c in range(4):
        xt[:, HALO * c:HALO * (c + 1)] = x_halo[:, 128 * c:128 * c + 128].T
    # xe: [128, 8*513], block m = rows [64m, 64m+128) with ones column
    xe_rows = np.concatenate(
        [x_halo, np.ones((HALO, 1), np.float32)], 1)    # [576, 513]
    xe = np.empty((128, 8 * 513), np.float32)
    for m in range(NBLK):
        xe[:, 513 * m:513 * (m + 1)] = xe_rows[64 * m:64 * m + 128, :]
    # wws: [128, 1147] = w[640] | wa_wide[124] | sh[255] | Rlo[32] | Rhi[32]
    #                    | spare[64]
    wws = np.zeros((128, 1147), np.float32)
    for c in range(4):
        wws[:, 128 * c:128 * c + 128] = np.tile(Wt[128 * c:128 * c + 128, :],
                                                (1, 4))
        wws[:, 512 + 32 * c:512 + 32 * c + 32] = Wx[128 * c:128 * c + 128, :]
    for g in range(4):
        wws[32 * g:32 * g + 32, 640 + 60 + g] = Wa[:, 0]
    kk = np.arange(128)
    wws[kk, 764 + kk + 127] = 1.0
    # edge-mask factors: E[d', t] += -30 where j = t0 + t + d' - 32 invalid.
    # left edge (t0 == 0):  invalid iff t + d' < 32  (d' = k in [0,32))
    # right edge (t0+512 == T): invalid iff t + d' > 543 (d' = k+32)
    ks = np.arange(32)[:, None]
    ts = np.arange(32)[None, :]
    if t0 == 0:
        wws[0:32, 1019:1051] = np.where(ts < 32 - ks, -30.0, 0.0)
    if t0 + T_LOC == T:
        wws[0:32, 1051:1083] = np.where((480 + ts) + (ks + 32) > 543,
                                        -30.0, 0.0)
    # mbb: [128, 2] = ba (rows 0-63) | bh4
    mbb = np.zeros((128, 2), np.float32)
    mbb[0:64, 0] = float(np.asarray(ba).reshape(-1)[0])
    mbb[:, 1] = np.tile(np.asarray(bh, np.float32), 4)

    return {
        "xt": xt.astype(ndt),
        "xe": xe.astype(ndt),
        "wws": wws.astype(ndt),
        "mbb": mbb,
    }


_NC_CACHE = {}


def _get_nc(cdt=_CDT):
    key = str(cdt)
    if key not in _NC_CACHE:
        _NC_CACHE[key] = build_nc(cdt)
    return _NC_CACHE[key]


def kernel(x, Wt, Wx, bh, Wa, ba, _trace=False):
    x = np.asarray(x, np.float32)
    Wt = np.asarray(Wt, np.float32)
    Wx = np.asarray(Wx, np.float32)
    bh = np.asarray(bh, np.float32)
    Wa = np.asarray(Wa, np.float32)
    ba = np.asarray(ba, np.float32)
    nc = _get_nc()
    in_maps = [prep_core_inputs(x, Wt, Wx, bh, Wa, ba, c)
               for c in range(NCORES)]
    res = run_bass_kernel_spmd(nc, in_maps, core_ids=list(range(NCORES)),
                               trace=_trace)
    out = np.empty((B, T, D), np.float32)
    for c in range(NCORES):
        b, half = c // 2, c % 2
        out[b, half * T_LOC:(half + 1) * T_LOC, :] = np.asarray(
            res.results[c]["v"], np.float32)
    if _trace:
        return out, res
    return out
